# revision 14
# baseline (speedup 1.0000x reference)
"""NMS-detection kernel for 8 TRN2 NeuronCores.

Reference computation: per aux channel c (32) and batch b (64), peak masks
(strict local maxima >= 20 over the channel-flattened [B,F,T] array) are
intersected with the strain peak mask; outputs are IoU and inter/|strain|
ratios, flattened to two [2048] vectors.

Strategy:
  - Shard qt_aux along N_aux: 4 channels per core.
  - Host precomputes the strain peak mask (replicated, per the sharding hint)
    as a bf16 0/1 tensor; host does the final tiny [32,64] divisions.
  - Device streams the 52 MB/core of aux data once (memory-bound), computing
    per chunk three 1x DVE ops:
        u   = max(left, THRESH_PRED) max right        (scalar_tensor_tensor)
        ma  = (u bypass) is_lt center, accum -> c2    (scalar_tensor_tensor)
        pr  = (ma mult 1) mult strain_mask, accum -> inter
    The fused accum_out gives per-partition sums for free; partition p covers
    a contiguous 25600-elem span so batch b = partitions {2b, 2b+1}.
"""

import numpy as np

N_AUX = 32
B, F, T = 64, 80, 640
N = B * F * T            # 3,276,800 elems per channel
P = 128                  # SBUF partitions
ROW = N // P             # 25,600 elems per partition row
N_CORES = 8
CH_PER_CORE = N_AUX // N_CORES   # 4
THRESHOLD = 20.0
PAD = np.float32(1.0e30)         # sentinel neighbor: kills first/last peaks
NCHUNK = 5
FCH = ROW // NCHUNK      # 5,120 elems per chunk

_CACHE = {}


def _build_bass():
    from concourse import bacc, mybir, tile

    nc = bacc.Bacc(None)
    f32 = mybir.dt.float32
    bf16 = mybir.dt.bfloat16
    Alu = mybir.AluOpType

    aux = nc.declare_dram_parameter("aux", [CH_PER_CORE, P, ROW + 2], f32,
                                    isOutput=False)
    msk = nc.declare_dram_parameter("msk", [P, ROW], bf16, isOutput=False)
    out = nc.declare_dram_parameter("out", [P, CH_PER_CORE * 2], f32,
                                    isOutput=True)

    # largest f32 strictly below THRESHOLD: (x >= 20.0) == (x > TP)
    TP = float(np.nextafter(np.float32(THRESHOLD), np.float32(0.0)))

    with tile.TileContext(nc) as tc:
        with (
            tc.tile_pool(name="res", bufs=1) as res_pool,
            tc.tile_pool(name="xin", bufs=3) as x_pool,
            tc.tile_pool(name="u", bufs=1) as u_pool,
            tc.tile_pool(name="ma", bufs=1) as ma_pool,
            tc.tile_pool(name="pr", bufs=2) as pr_pool,
            tc.tile_pool(name="acc", bufs=2) as acc_pool,
        ):
            ms = res_pool.tile([P, ROW], bf16)
            outt = res_pool.tile([P, CH_PER_CORE * 2], f32)

            for c in range(CH_PER_CORE):
                acc_i = acc_pool.tile([P, NCHUNK], f32, tag="acc_i")
                acc_c = acc_pool.tile([P, NCHUNK], f32, tag="acc_c")
                for j in range(NCHUNK):
                    if c == 0:
                        # stream the strain mask in slices so chunk 0's
                        # multiply isn't gated on the full 6.5 MB load
                        nc.sync.dma_start(
                            out=ms[:, j * FCH:(j + 1) * FCH],
                            in_=msk[:, j * FCH:(j + 1) * FCH])
                    x = x_pool.tile([P, FCH + 2], f32)
                    # gpsimd = SWDGE: waits execute on the engine, so the
                    # WAR+WAW pair on slot reuse doesn't overflow the single
                    # HW-DMA descriptor wait slot.
                    nc.gpsimd.dma_start(
                        out=x[:], in_=aux[c, :, j * FCH: j * FCH + FCH + 2])
                    u = u_pool.tile([P, FCH], f32)
                    # u = max(left, TP, right)
                    nc.vector.scalar_tensor_tensor(
                        u[:], x[:, 0:FCH], TP, x[:, 2:FCH + 2],
                        op0=Alu.max, op1=Alu.max)
                    ma = ma_pool.tile([P, FCH], bf16)
                    # ma = (u < center) ; acc_c[:, j] = sum(ma)
                    nc.vector.scalar_tensor_tensor(
                        ma[:], u[:], 0.0, x[:, 1:FCH + 1],
                        op0=Alu.bypass, op1=Alu.is_lt,
                        accum_out=acc_c[:, j:j + 1])
                    pr = pr_pool.tile([P, FCH], bf16)
                    # pr = ma * ms   (tensor_tensor bf16 -> 2x mode)
                    nc.vector.tensor_tensor(
                        pr[:], ma[:], ms[:, j * FCH:(j + 1) * FCH],
                        op=Alu.mult)
                    # acc_i[:, j] = sum(pr)   (tensor_scalar bf16 -> 4x mode)
                    nc.vector.tensor_scalar(
                        pr[:], pr[:], 1.0, None,
                        op0=Alu.mult, op1=Alu.add,
                        accum_out=acc_i[:, j:j + 1])
                nc.vector.tensor_reduce(
                    outt[:, 2 * c:2 * c + 1], acc_i[:],
                    axis=mybir.AxisListType.X, op=Alu.add)
                nc.vector.tensor_reduce(
                    outt[:, 2 * c + 1:2 * c + 2], acc_c[:],
                    axis=mybir.AxisListType.X, op=Alu.add)

            nc.sync.dma_start(out=out[:], in_=outt[:])

    _prune_redundant_dma_waits(nc, mybir)
    # Bacc compile splits remaining multi-wait instructions (TRN2 allows one
    # sync wait per instruction) via event semaphores, allocs regs, etc.
    nc.compile()
    return nc


def _prune_redundant_dma_waits(nc, mybir):
    """Walrus rejects DMA descriptors with >1 sync wait. Tile (with
    optimize_sems disabled) emits WAR+WAW wait pairs on slot-reusing loads,
    where the WAW wait on the previous DMA's sem is transitively implied by
    the WAR wait (the reader already waited on that DMA). Drop exactly those
    provably-implied DMA-sem waits.

    Soundness: sem S >= v implies every instruction whose update brought S to
    a cumulative value <= v has completed, hence its own waits held. We
    propagate that knowledge (sem -> max implied value) per increment.
    """
    know = {}   # (sem_name, cum_value) -> dict{sem_name: max_value}
    last_ticks = {}  # sem_name -> list of cum values recorded
    cum = {}

    def lookup(sem, val):
        ticks = last_ticks.get(sem)
        if not ticks:
            return {}
        # largest recorded cum tick <= val
        best = None
        for t in ticks:
            if t <= val:
                best = t
            else:
                break
        return know.get((sem, best), {}) if best is not None else {}

    def merge(dst, src):
        for k, v in src.items():
            if dst.get(k, -1) < v:
                dst[k] = v

    insts = [i for b in nc.m.functions[0].blocks for i in b.instructions]
    for inst in insts:
        si = inst.sync_info
        if si is None:
            continue
        waits = list(si.on_wait or [])
        updates = list(si.on_update or [])
        if updates:
            k = {}
            for w in waits:
                if w.wait_value is None:
                    continue
                k[w.ant_name] = max(k.get(w.ant_name, -1), w.wait_value)
                merge(k, lookup(w.ant_name, w.wait_value))
            for u in updates:
                if u.update_value is None:
                    continue
                c = cum.get(u.ant_name, 0) + u.update_value
                cum[u.ant_name] = c
                prev = lookup(u.ant_name, c)
                kk = dict(prev)
                merge(kk, k)
                know[(u.ant_name, c)] = kk
                last_ticks.setdefault(u.ant_name, []).append(c)

    for inst in insts:
        if inst.opcode != "DMACopy":
            continue
        si = inst.sync_info
        if si is None or not si.on_wait or len(si.on_wait) <= 1:
            continue
        waits = list(si.on_wait)
        kept = []
        for i, w in enumerate(waits):
            if not (w.ant_name.startswith("DMASW")
                    or w.ant_name.startswith("DMAHW")):
                kept.append(w)
                continue
            implied = False
            for j, w2 in enumerate(waits):
                if j == i:
                    continue
                if lookup(w2.ant_name, w2.wait_value).get(w.ant_name, -1) \
                        >= w.wait_value:
                    implied = True
                    break
            if not implied:
                kept.append(w)
        if len(kept) > 1:
            raise RuntimeError(
                f"{inst.name}: DMA still has {len(kept)} waits after pruning: "
                f"{[(w.ant_name, w.wait_value) for w in kept]}")
        inst.sync_info = mybir.SyncInfo(on_wait=kept, on_update=list(si.on_update))


def _get_nc():
    if "nc" not in _CACHE:
        _CACHE["nc"] = _build_bass()
    return _CACHE["nc"]


def _host_strain_mask(qt_strain):
    x = np.ascontiguousarray(qt_strain, dtype=np.float32).reshape(-1)
    m = np.zeros(N, dtype=bool)
    m[1:-1] = (x[1:-1] > x[:-2]) & (x[1:-1] > x[2:]) & (x[1:-1] >= THRESHOLD)
    return m


def _prep_inputs(qt_strain, qt_aux):
    import ml_dtypes
    ms_flat = _host_strain_mask(qt_strain)
    c1 = ms_flat.reshape(B, F * T).sum(axis=1).astype(np.int64)   # [64]
    ms_dev = ms_flat.reshape(P, ROW).astype(ml_dtypes.bfloat16)

    afl = np.ascontiguousarray(qt_aux, dtype=np.float32).reshape(N_AUX, N)
    apad = np.empty((N_AUX, N + 2), np.float32)
    apad[:, 0] = PAD
    apad[:, -1] = PAD
    apad[:, 1:-1] = afl
    sv = np.lib.stride_tricks.as_strided(
        apad, shape=(N_AUX, P, ROW + 2),
        strides=(apad.strides[0], ROW * 4, 4))
    aux_dev = np.ascontiguousarray(sv)     # [32, 128, 25602]
    in_maps = [
        {"aux": aux_dev[i * CH_PER_CORE:(i + 1) * CH_PER_CORE], "msk": ms_dev}
        for i in range(N_CORES)
    ]
    return in_maps, c1


def _postprocess(results, c1):
    inter = np.empty((N_AUX, B), np.int64)
    c2 = np.empty((N_AUX, B), np.int64)
    for i in range(N_CORES):
        o = np.asarray(results[i]["out"], dtype=np.float64)   # [128, 8]
        for c in range(CH_PER_CORE):
            ch = i * CH_PER_CORE + c
            inter[ch] = np.rint(o[:, 2 * c].reshape(B, 2).sum(axis=1))
            c2[ch] = np.rint(o[:, 2 * c + 1].reshape(B, 2).sum(axis=1))

    interf = inter.astype(np.float32)
    c2f = c2.astype(np.float32)
    c1f = np.broadcast_to(c1.astype(np.float32), (N_AUX, B))
    union = c1f + c2f - interf
    with np.errstate(divide="ignore", invalid="ignore"):
        jac = interf / union
        ratio = interf / c1f
    zero_union = (interf == 0) & (union == 0)
    jac = np.where(zero_union, np.float32(1.0), jac)
    ratio = np.where(zero_union, np.float32(1.0), ratio)
    jac = np.nan_to_num(jac, nan=0.0)
    ratio = np.nan_to_num(ratio, nan=0.0)
    return (jac.reshape(-1).astype(np.float32),
            ratio.reshape(-1).astype(np.float32))


def _run(qt_strain, qt_aux, trace=False, **kw):
    from concourse.bass_utils import run_bass_kernel_spmd
    nc = _get_nc()
    in_maps, c1 = _prep_inputs(qt_strain, qt_aux)
    res = run_bass_kernel_spmd(nc, in_maps, list(range(N_CORES)),
                               trace=trace, **kw)
    return _postprocess(res.results, c1), res


def kernel(qt_strain, qt_aux):
    out, _ = _run(qt_strain, qt_aux, trace=False)
    return out


# revision 19
# speedup vs baseline: 1.3194x; 1.3194x over previous
"""NMS-detection kernel for 8 TRN2 NeuronCores.

Reference computation: per aux channel c (32) and batch b (64), peak masks
(strict local maxima >= 20 over the channel-flattened [B,F,T] array) are
intersected with the strain peak mask; outputs are IoU and inter/|strain|
ratios, flattened to two [2048] vectors.

Strategy:
  - Shard qt_aux along N_aux: 4 channels per core.
  - Host precomputes the strain peak mask (replicated, per the sharding hint)
    as a bf16 0/1 tensor; host does the final tiny [32,64] divisions.
  - Device streams the 52 MB/core of aux data once (memory-bound), computing
    per chunk three 1x DVE ops:
        u   = max(left, THRESH_PRED) max right        (scalar_tensor_tensor)
        ma  = (u bypass) is_lt center, accum -> c2    (scalar_tensor_tensor)
        pr  = (ma mult 1) mult strain_mask, accum -> inter
    The fused accum_out gives per-partition sums for free; partition p covers
    a contiguous 25600-elem span so batch b = partitions {2b, 2b+1}.
"""

import numpy as np

N_AUX = 32
B, F, T = 64, 80, 640
N = B * F * T            # 3,276,800 elems per channel
P = 128                  # SBUF partitions
ROW = N // P             # 25,600 elems per partition row
N_CORES = 8
CH_PER_CORE = N_AUX // N_CORES   # 4
THRESHOLD = 20.0
PAD = np.float32(1.0e30)         # sentinel neighbor: kills first/last peaks
NCHUNK = 5
FCH = ROW // NCHUNK      # 5,120 elems per chunk

_CACHE = {}


def _build_bass():
    from concourse import bacc, mybir, tile

    nc = bacc.Bacc(None)
    f32 = mybir.dt.float32
    bf16 = mybir.dt.bfloat16
    Alu = mybir.AluOpType

    aux = nc.declare_dram_parameter("aux", [CH_PER_CORE, P, ROW + 2], f32,
                                    isOutput=False)
    msk = nc.declare_dram_parameter("msk", [P, ROW], bf16, isOutput=False)
    selp = nc.declare_dram_parameter("sel", [P, B], bf16, isOutput=False)
    out = nc.declare_dram_parameter("out", [P, CH_PER_CORE * 2], f32,
                                    isOutput=True)
    MMB = 512

    # largest f32 strictly below THRESHOLD: (x >= 20.0) == (x > TP)
    TP = float(np.nextafter(np.float32(THRESHOLD), np.float32(0.0)))

    with tile.TileContext(nc) as tc:
        with (
            tc.tile_pool(name="res", bufs=1) as res_pool,
            tc.tile_pool(name="xin", bufs=3) as x_pool,
            tc.tile_pool(name="u", bufs=1) as u_pool,
            tc.tile_pool(name="ma", bufs=1) as ma_pool,
            tc.tile_pool(name="pr", bufs=2) as pr_pool,
            tc.tile_pool(name="acc", bufs=2) as acc_pool,
            tc.tile_pool(name="ps", bufs=2, space="PSUM") as psum_pool,
        ):
            ms = res_pool.tile([P, ROW], bf16)
            sel = res_pool.tile([P, B], bf16)
            nc.sync.dma_start(out=sel[:], in_=selp[:])
            outt = res_pool.tile([P, CH_PER_CORE * 2], f32)

            for c in range(CH_PER_CORE):
                psum = psum_pool.tile([B, MMB], f32)
                acc_c = acc_pool.tile([P, NCHUNK], f32, tag="acc_c")
                for j in range(NCHUNK):
                    if c == 0:
                        # stream the strain mask in slices so chunk 0's
                        # multiply isn't gated on the full 6.5 MB load
                        nc.sync.dma_start(
                            out=ms[:, j * FCH:(j + 1) * FCH],
                            in_=msk[:, j * FCH:(j + 1) * FCH])
                    x = x_pool.tile([P, FCH + 2], f32)
                    # gpsimd = SWDGE: waits execute on the engine, so the
                    # WAR+WAW pair on slot reuse doesn't overflow the single
                    # HW-DMA descriptor wait slot.
                    nc.gpsimd.dma_start(
                        out=x[:], in_=aux[c, :, j * FCH: j * FCH + FCH + 2])
                    u = u_pool.tile([P, FCH], f32)
                    # u = max(left, TP, right)
                    nc.vector.scalar_tensor_tensor(
                        u[:], x[:, 0:FCH], TP, x[:, 2:FCH + 2],
                        op0=Alu.max, op1=Alu.max)
                    ma = ma_pool.tile([P, FCH], bf16)
                    # ma = (u < center) ; acc_c[:, j] = sum(ma)
                    nc.vector.scalar_tensor_tensor(
                        ma[:], u[:], 0.0, x[:, 1:FCH + 1],
                        op0=Alu.bypass, op1=Alu.is_lt,
                        accum_out=acc_c[:, j:j + 1])
                    pr = pr_pool.tile([P, FCH], bf16)
                    # pr = ma * ms   (tensor_tensor bf16 -> 2x mode)
                    nc.vector.tensor_tensor(
                        pr[:], ma[:], ms[:, j * FCH:(j + 1) * FCH],
                        op=Alu.mult)
                    # inter reduction on TensorE: psum[b, :] += sel.T @ pr
                    for s in range(FCH // MMB):
                        nc.tensor.matmul(
                            psum[:, :], sel[:],
                            pr[:, s * MMB:(s + 1) * MMB],
                            start=(j == 0 and s == 0),
                            stop=(j == NCHUNK - 1 and s == FCH // MMB - 1))
                nc.vector.tensor_reduce(
                    outt[0:B, 2 * c:2 * c + 1], psum[:],
                    axis=mybir.AxisListType.X, op=Alu.add)
                nc.vector.tensor_reduce(
                    outt[:, 2 * c + 1:2 * c + 2], acc_c[:],
                    axis=mybir.AxisListType.X, op=Alu.add)

            nc.sync.dma_start(out=out[:], in_=outt[:])

    _prune_redundant_dma_waits(nc, mybir)
    # Bacc compile splits remaining multi-wait instructions (TRN2 allows one
    # sync wait per instruction) via event semaphores, allocs regs, etc.
    nc.compile()
    return nc


def _prune_redundant_dma_waits(nc, mybir):
    """Walrus rejects DMA descriptors with >1 sync wait. Tile (with
    optimize_sems disabled) emits WAR+WAW wait pairs on slot-reusing loads,
    where the WAW wait on the previous DMA's sem is transitively implied by
    the WAR wait (the reader already waited on that DMA). Drop exactly those
    provably-implied DMA-sem waits.

    Soundness: sem S >= v implies every instruction whose update brought S to
    a cumulative value <= v has completed, hence its own waits held. We
    propagate that knowledge (sem -> max implied value) per increment.
    """
    know = {}   # (sem_name, cum_value) -> dict{sem_name: max_value}
    last_ticks = {}  # sem_name -> list of cum values recorded
    cum = {}

    def lookup(sem, val):
        ticks = last_ticks.get(sem)
        if not ticks:
            return {}
        # largest recorded cum tick <= val
        best = None
        for t in ticks:
            if t <= val:
                best = t
            else:
                break
        return know.get((sem, best), {}) if best is not None else {}

    def merge(dst, src):
        for k, v in src.items():
            if dst.get(k, -1) < v:
                dst[k] = v

    insts = [i for b in nc.m.functions[0].blocks for i in b.instructions]
    for inst in insts:
        si = inst.sync_info
        if si is None:
            continue
        waits = list(si.on_wait or [])
        updates = list(si.on_update or [])
        if updates:
            k = {}
            for w in waits:
                if w.wait_value is None:
                    continue
                k[w.ant_name] = max(k.get(w.ant_name, -1), w.wait_value)
                merge(k, lookup(w.ant_name, w.wait_value))
            for u in updates:
                if u.update_value is None:
                    continue
                c = cum.get(u.ant_name, 0) + u.update_value
                cum[u.ant_name] = c
                prev = lookup(u.ant_name, c)
                kk = dict(prev)
                merge(kk, k)
                know[(u.ant_name, c)] = kk
                last_ticks.setdefault(u.ant_name, []).append(c)

    for inst in insts:
        if inst.opcode != "DMACopy":
            continue
        si = inst.sync_info
        if si is None or not si.on_wait or len(si.on_wait) <= 1:
            continue
        waits = list(si.on_wait)
        kept = []
        for i, w in enumerate(waits):
            if not (w.ant_name.startswith("DMASW")
                    or w.ant_name.startswith("DMAHW")):
                kept.append(w)
                continue
            implied = False
            for j, w2 in enumerate(waits):
                if j == i:
                    continue
                if lookup(w2.ant_name, w2.wait_value).get(w.ant_name, -1) \
                        >= w.wait_value:
                    implied = True
                    break
            if not implied:
                kept.append(w)
        if len(kept) > 1:
            raise RuntimeError(
                f"{inst.name}: DMA still has {len(kept)} waits after pruning: "
                f"{[(w.ant_name, w.wait_value) for w in kept]}")
        inst.sync_info = mybir.SyncInfo(on_wait=kept, on_update=list(si.on_update))


def _get_nc():
    if "nc" not in _CACHE:
        _CACHE["nc"] = _build_bass()
    return _CACHE["nc"]


def _host_strain_mask(qt_strain):
    x = np.ascontiguousarray(qt_strain, dtype=np.float32).reshape(-1)
    m = np.zeros(N, dtype=bool)
    m[1:-1] = (x[1:-1] > x[:-2]) & (x[1:-1] > x[2:]) & (x[1:-1] >= THRESHOLD)
    return m


def _prep_inputs(qt_strain, qt_aux):
    import ml_dtypes
    ms_flat = _host_strain_mask(qt_strain)
    c1 = ms_flat.reshape(B, F * T).sum(axis=1).astype(np.int64)   # [64]
    ms_dev = ms_flat.reshape(P, ROW).astype(ml_dtypes.bfloat16)

    afl = np.ascontiguousarray(qt_aux, dtype=np.float32).reshape(N_AUX, N)
    apad = np.empty((N_AUX, N + 2), np.float32)
    apad[:, 0] = PAD
    apad[:, -1] = PAD
    apad[:, 1:-1] = afl
    sv = np.lib.stride_tricks.as_strided(
        apad, shape=(N_AUX, P, ROW + 2),
        strides=(apad.strides[0], ROW * 4, 4))
    aux_dev = np.ascontiguousarray(sv)     # [32, 128, 25602]
    # sel[p, b] = 1 if partition p belongs to batch b (p in {2b, 2b+1})
    sel_dev = (np.arange(P)[:, None] // 2 ==
               np.arange(B)[None, :]).astype(ml_dtypes.bfloat16)
    in_maps = [
        {"aux": aux_dev[i * CH_PER_CORE:(i + 1) * CH_PER_CORE],
         "msk": ms_dev, "sel": sel_dev}
        for i in range(N_CORES)
    ]
    return in_maps, c1


def _postprocess(results, c1):
    inter = np.empty((N_AUX, B), np.int64)
    c2 = np.empty((N_AUX, B), np.int64)
    for i in range(N_CORES):
        o = np.asarray(results[i]["out"], dtype=np.float64)   # [128, 8]
        for c in range(CH_PER_CORE):
            ch = i * CH_PER_CORE + c
            inter[ch] = np.rint(o[0:B, 2 * c])
            c2[ch] = np.rint(o[:, 2 * c + 1].reshape(B, 2).sum(axis=1))

    interf = inter.astype(np.float32)
    c2f = c2.astype(np.float32)
    c1f = np.broadcast_to(c1.astype(np.float32), (N_AUX, B))
    union = c1f + c2f - interf
    with np.errstate(divide="ignore", invalid="ignore"):
        jac = interf / union
        ratio = interf / c1f
    zero_union = (interf == 0) & (union == 0)
    jac = np.where(zero_union, np.float32(1.0), jac)
    ratio = np.where(zero_union, np.float32(1.0), ratio)
    jac = np.nan_to_num(jac, nan=0.0)
    ratio = np.nan_to_num(ratio, nan=0.0)
    return (jac.reshape(-1).astype(np.float32),
            ratio.reshape(-1).astype(np.float32))


def _run(qt_strain, qt_aux, trace=False, **kw):
    from concourse.bass_utils import run_bass_kernel_spmd
    nc = _get_nc()
    in_maps, c1 = _prep_inputs(qt_strain, qt_aux)
    res = run_bass_kernel_spmd(nc, in_maps, list(range(N_CORES)),
                               trace=trace, **kw)
    return _postprocess(res.results, c1), res


def kernel(qt_strain, qt_aux):
    out, _ = _run(qt_strain, qt_aux, trace=False)
    return out


# revision 21
# speedup vs baseline: 1.3362x; 1.0127x over previous
"""NMS-detection kernel for 8 TRN2 NeuronCores.

Reference computation: per aux channel c (32) and batch b (64), peak masks
(strict local maxima >= 20 over the channel-flattened [B,F,T] array) are
intersected with the strain peak mask; outputs are IoU and inter/|strain|
ratios, flattened to two [2048] vectors.

Strategy:
  - Shard qt_aux along N_aux: 4 channels per core.
  - Host precomputes the strain peak mask (replicated, per the sharding hint)
    as a bf16 0/1 tensor; host does the final tiny [32,64] divisions.
  - Device streams the 52 MB/core of aux data once (memory-bound), computing
    per chunk three 1x DVE ops:
        u   = max(left, THRESH_PRED) max right        (scalar_tensor_tensor)
        ma  = (u bypass) is_lt center, accum -> c2    (scalar_tensor_tensor)
        pr  = (ma mult 1) mult strain_mask, accum -> inter
    The fused accum_out gives per-partition sums for free; partition p covers
    a contiguous 25600-elem span so batch b = partitions {2b, 2b+1}.
"""

import numpy as np

N_AUX = 32
B, F, T = 64, 80, 640
N = B * F * T            # 3,276,800 elems per channel
P = 128                  # SBUF partitions
ROW = N // P             # 25,600 elems per partition row
N_CORES = 8
CH_PER_CORE = N_AUX // N_CORES   # 4
THRESHOLD = 20.0
PAD = np.float32(1.0e30)         # sentinel neighbor: kills first/last peaks
NCHUNK = 5
FCH = ROW // NCHUNK      # 5,120 elems per chunk

_CACHE = {}


def _build_bass():
    from concourse import bacc, mybir, tile

    nc = bacc.Bacc(None)
    f32 = mybir.dt.float32
    bf16 = mybir.dt.bfloat16
    Alu = mybir.AluOpType

    aux = nc.declare_dram_parameter("aux", [CH_PER_CORE, P, ROW + 2], f32,
                                    isOutput=False)
    msk = nc.declare_dram_parameter("msk", [P, ROW], bf16, isOutput=False)
    selp = nc.declare_dram_parameter("sel", [P, B], bf16, isOutput=False)
    out = nc.declare_dram_parameter("out", [P, CH_PER_CORE * 2], f32,
                                    isOutput=True)
    MMB = 512

    # largest f32 strictly below THRESHOLD: (x >= 20.0) == (x > TP)
    TP = float(np.nextafter(np.float32(THRESHOLD), np.float32(0.0)))

    with tile.TileContext(nc) as tc:
        with (
            tc.tile_pool(name="res", bufs=1) as res_pool,
            tc.tile_pool(name="xin", bufs=3) as x_pool,
            tc.tile_pool(name="u", bufs=1) as u_pool,
            tc.tile_pool(name="ma", bufs=1) as ma_pool,
            tc.tile_pool(name="pr", bufs=2) as pr_pool,
            tc.tile_pool(name="acc", bufs=2) as acc_pool,
            tc.tile_pool(name="ps", bufs=2, space="PSUM") as psum_pool,
        ):
            ms = res_pool.tile([P, ROW], bf16)
            sel = res_pool.tile([P, B], bf16)
            nc.sync.dma_start(out=sel[:], in_=selp[:])
            outt = res_pool.tile([P, CH_PER_CORE * 2], f32)

            # half-chunk grid: the final piece of each channel is two
            # halves so the tail matmul chain runs on half the data
            pieces = [(j * FCH, FCH) for j in range(NCHUNK - 1)]
            pieces += [((NCHUNK - 1) * FCH, FCH // 2),
                       ((NCHUNK - 1) * FCH + FCH // 2, FCH // 2)]

            for c in range(CH_PER_CORE):
                psum = psum_pool.tile([B, MMB], f32)
                acc_c = acc_pool.tile([P, len(pieces)], f32, tag="acc_c")
                for j, (off, ln) in enumerate(pieces):
                    x = x_pool.tile([P, FCH + 2], f32, tag="x")
                    nc.sync.dma_start(
                        out=x[:, 0:ln + 2], in_=aux[c, :, off: off + ln + 2])
                    if c == 0:
                        # stream the strain mask in slices so chunk 0's
                        # multiply isn't gated on the full 6.5 MB load
                        nc.sync.dma_start(
                            out=ms[:, off:off + ln], in_=msk[:, off:off + ln])
                    u = u_pool.tile([P, FCH], f32, tag="u")
                    # u = max(left, TP, right)
                    nc.vector.scalar_tensor_tensor(
                        u[:, 0:ln], x[:, 0:ln], TP, x[:, 2:ln + 2],
                        op0=Alu.max, op1=Alu.max)
                    ma = ma_pool.tile([P, FCH], bf16, tag="ma")
                    # ma = (u < center) ; acc_c[:, j] = sum(ma)
                    nc.vector.scalar_tensor_tensor(
                        ma[:, 0:ln], u[:, 0:ln], 0.0, x[:, 1:ln + 1],
                        op0=Alu.bypass, op1=Alu.is_lt,
                        accum_out=acc_c[:, j:j + 1])
                    pr = pr_pool.tile([P, FCH], bf16, tag="pr")
                    # pr = ma * ms   (tensor_tensor bf16 -> 2x mode)
                    nc.vector.tensor_tensor(
                        pr[:, 0:ln], ma[:, 0:ln], ms[:, off:off + ln],
                        op=Alu.mult)
                    # inter reduction on TensorE: psum[b, :] += sel.T @ pr
                    for s in range(ln // MMB):
                        nc.tensor.matmul(
                            psum[:, :], sel[:],
                            pr[:, s * MMB:(s + 1) * MMB],
                            start=(j == 0 and s == 0),
                            stop=(j == len(pieces) - 1
                                  and s == ln // MMB - 1))
                nc.vector.tensor_reduce(
                    outt[0:B, 2 * c:2 * c + 1], psum[:],
                    axis=mybir.AxisListType.X, op=Alu.add)
                nc.vector.tensor_reduce(
                    outt[:, 2 * c + 1:2 * c + 2], acc_c[:],
                    axis=mybir.AxisListType.X, op=Alu.add)
                # ship each channel's two columns as soon as they're ready
                nc.sync.dma_start(out=out[:, 2 * c:2 * c + 2],
                                  in_=outt[:, 2 * c:2 * c + 2])

    _prune_redundant_dma_waits(nc, mybir)
    # Bacc compile splits remaining multi-wait instructions (TRN2 allows one
    # sync wait per instruction) via event semaphores, allocs regs, etc.
    nc.compile()
    return nc


def _prune_redundant_dma_waits(nc, mybir):
    """Walrus rejects DMA descriptors with >1 sync wait. Tile (with
    optimize_sems disabled) emits WAR+WAW wait pairs on slot-reusing loads,
    where the WAW wait on the previous DMA's sem is transitively implied by
    the WAR wait (the reader already waited on that DMA). Drop exactly those
    provably-implied DMA-sem waits.

    Soundness: sem S >= v implies every instruction whose update brought S to
    a cumulative value <= v has completed, hence its own waits held. We
    propagate that knowledge (sem -> max implied value) per increment.
    """
    know = {}   # (sem_name, cum_value) -> dict{sem_name: max_value}
    last_ticks = {}  # sem_name -> list of cum values recorded
    cum = {}

    def lookup(sem, val):
        ticks = last_ticks.get(sem)
        if not ticks:
            return {}
        # largest recorded cum tick <= val
        best = None
        for t in ticks:
            if t <= val:
                best = t
            else:
                break
        return know.get((sem, best), {}) if best is not None else {}

    def merge(dst, src):
        for k, v in src.items():
            if dst.get(k, -1) < v:
                dst[k] = v

    insts = [i for b in nc.m.functions[0].blocks for i in b.instructions]
    for inst in insts:
        si = inst.sync_info
        if si is None:
            continue
        waits = list(si.on_wait or [])
        updates = list(si.on_update or [])
        if updates:
            k = {}
            for w in waits:
                if w.wait_value is None:
                    continue
                k[w.ant_name] = max(k.get(w.ant_name, -1), w.wait_value)
                merge(k, lookup(w.ant_name, w.wait_value))
            for u in updates:
                if u.update_value is None:
                    continue
                c = cum.get(u.ant_name, 0) + u.update_value
                cum[u.ant_name] = c
                prev = lookup(u.ant_name, c)
                kk = dict(prev)
                merge(kk, k)
                know[(u.ant_name, c)] = kk
                last_ticks.setdefault(u.ant_name, []).append(c)

    for inst in insts:
        if inst.opcode != "DMACopy":
            continue
        si = inst.sync_info
        if si is None or not si.on_wait or len(si.on_wait) <= 1:
            continue
        waits = list(si.on_wait)
        kept = []
        for i, w in enumerate(waits):
            if not (w.ant_name.startswith("DMASW")
                    or w.ant_name.startswith("DMAHW")):
                kept.append(w)
                continue
            implied = False
            for j, w2 in enumerate(waits):
                if j == i:
                    continue
                if lookup(w2.ant_name, w2.wait_value).get(w.ant_name, -1) \
                        >= w.wait_value:
                    implied = True
                    break
            if not implied:
                kept.append(w)
        # any instruction still multi-wait is split legally by Bacc's
        # generate_event_semaphores during nc.compile()
        inst.sync_info = mybir.SyncInfo(on_wait=kept, on_update=list(si.on_update))


def _get_nc():
    if "nc" not in _CACHE:
        _CACHE["nc"] = _build_bass()
    return _CACHE["nc"]


def _host_strain_mask(qt_strain):
    x = np.ascontiguousarray(qt_strain, dtype=np.float32).reshape(-1)
    m = np.zeros(N, dtype=bool)
    m[1:-1] = (x[1:-1] > x[:-2]) & (x[1:-1] > x[2:]) & (x[1:-1] >= THRESHOLD)
    return m


def _prep_inputs(qt_strain, qt_aux):
    import ml_dtypes
    ms_flat = _host_strain_mask(qt_strain)
    c1 = ms_flat.reshape(B, F * T).sum(axis=1).astype(np.int64)   # [64]
    ms_dev = ms_flat.reshape(P, ROW).astype(ml_dtypes.bfloat16)

    afl = np.ascontiguousarray(qt_aux, dtype=np.float32).reshape(N_AUX, N)
    apad = np.empty((N_AUX, N + 2), np.float32)
    apad[:, 0] = PAD
    apad[:, -1] = PAD
    apad[:, 1:-1] = afl
    sv = np.lib.stride_tricks.as_strided(
        apad, shape=(N_AUX, P, ROW + 2),
        strides=(apad.strides[0], ROW * 4, 4))
    aux_dev = np.ascontiguousarray(sv)     # [32, 128, 25602]
    # sel[p, b] = 1 if partition p belongs to batch b (p in {2b, 2b+1})
    sel_dev = (np.arange(P)[:, None] // 2 ==
               np.arange(B)[None, :]).astype(ml_dtypes.bfloat16)
    in_maps = [
        {"aux": aux_dev[i * CH_PER_CORE:(i + 1) * CH_PER_CORE],
         "msk": ms_dev, "sel": sel_dev}
        for i in range(N_CORES)
    ]
    return in_maps, c1


def _postprocess(results, c1):
    inter = np.empty((N_AUX, B), np.int64)
    c2 = np.empty((N_AUX, B), np.int64)
    for i in range(N_CORES):
        o = np.asarray(results[i]["out"], dtype=np.float64)   # [128, 8]
        for c in range(CH_PER_CORE):
            ch = i * CH_PER_CORE + c
            inter[ch] = np.rint(o[0:B, 2 * c])
            c2[ch] = np.rint(o[:, 2 * c + 1].reshape(B, 2).sum(axis=1))

    interf = inter.astype(np.float32)
    c2f = c2.astype(np.float32)
    c1f = np.broadcast_to(c1.astype(np.float32), (N_AUX, B))
    union = c1f + c2f - interf
    with np.errstate(divide="ignore", invalid="ignore"):
        jac = interf / union
        ratio = interf / c1f
    zero_union = (interf == 0) & (union == 0)
    jac = np.where(zero_union, np.float32(1.0), jac)
    ratio = np.where(zero_union, np.float32(1.0), ratio)
    jac = np.nan_to_num(jac, nan=0.0)
    ratio = np.nan_to_num(ratio, nan=0.0)
    return (jac.reshape(-1).astype(np.float32),
            ratio.reshape(-1).astype(np.float32))


def _run(qt_strain, qt_aux, trace=False, **kw):
    from concourse.bass_utils import run_bass_kernel_spmd
    nc = _get_nc()
    in_maps, c1 = _prep_inputs(qt_strain, qt_aux)
    res = run_bass_kernel_spmd(nc, in_maps, list(range(N_CORES)),
                               trace=trace, **kw)
    return _postprocess(res.results, c1), res


def kernel(qt_strain, qt_aux):
    out, _ = _run(qt_strain, qt_aux, trace=False)
    return out


# revision 24
# speedup vs baseline: 1.3552x; 1.0142x over previous
"""NMS-detection kernel for 8 TRN2 NeuronCores.

Reference computation: per aux channel c (32) and batch b (64), peak masks
(strict local maxima >= 20 over the channel-flattened [B,F,T] array) are
intersected with the strain peak mask; outputs are IoU and inter/|strain|
ratios, flattened to two [2048] vectors.

Strategy:
  - Shard qt_aux along N_aux: 4 channels per core.
  - Host precomputes the strain peak mask (replicated, per the sharding hint)
    as a bf16 0/1 tensor; host does the final tiny [32,64] divisions.
  - Device streams the 52 MB/core of aux data once (memory-bound), computing
    per chunk three 1x DVE ops:
        u   = max(left, THRESH_PRED) max right        (scalar_tensor_tensor)
        ma  = (u bypass) is_lt center, accum -> c2    (scalar_tensor_tensor)
        pr  = (ma mult 1) mult strain_mask, accum -> inter
    The fused accum_out gives per-partition sums for free; partition p covers
    a contiguous 25600-elem span so batch b = partitions {2b, 2b+1}.
"""

import numpy as np

N_AUX = 32
B, F, T = 64, 80, 640
N = B * F * T            # 3,276,800 elems per channel
P = 128                  # SBUF partitions
ROW = N // P             # 25,600 elems per partition row
N_CORES = 8
CH_PER_CORE = N_AUX // N_CORES   # 4
THRESHOLD = 20.0
PAD = np.float32(1.0e30)         # sentinel neighbor: kills first/last peaks
NCHUNK = 5
FCH = ROW // NCHUNK      # 5,120 elems per chunk

_CACHE = {}


def _build_bass():
    from concourse import bacc, mybir, tile

    nc = bacc.Bacc(None)
    f32 = mybir.dt.float32
    bf16 = mybir.dt.bfloat16
    Alu = mybir.AluOpType

    aux = nc.declare_dram_parameter("aux", [CH_PER_CORE, P, ROW + 2], f32,
                                    isOutput=False)
    msk = nc.declare_dram_parameter("msk", [P, ROW], bf16, isOutput=False)
    selp = nc.declare_dram_parameter("sel", [P, B], bf16, isOutput=False)
    out = nc.declare_dram_parameter("out", [P, CH_PER_CORE * 2], f32,
                                    isOutput=True)
    MMB = 512

    # largest f32 strictly below THRESHOLD: (x >= 20.0) == (x > TP)
    TP = float(np.nextafter(np.float32(THRESHOLD), np.float32(0.0)))

    with tile.TileContext(nc) as tc:
        with (
            tc.tile_pool(name="res", bufs=1) as res_pool,
            tc.tile_pool(name="xin", bufs=3) as x_pool,
            tc.tile_pool(name="u", bufs=1) as u_pool,
            tc.tile_pool(name="ma", bufs=1) as ma_pool,
            tc.tile_pool(name="pr", bufs=2) as pr_pool,
            tc.tile_pool(name="acc", bufs=2) as acc_pool,
            tc.tile_pool(name="ps", bufs=2, space="PSUM") as psum_pool,
        ):
            ms = res_pool.tile([P, ROW], bf16)
            sel = res_pool.tile([P, B], bf16)
            nc.sync.dma_start(out=sel[:], in_=selp[:])
            outt = res_pool.tile([P, CH_PER_CORE * 2], f32)

            # ramped piece grid: small leading pieces so the first compute
            # starts as soon as ~0.7 MB has landed; small trailing pieces so
            # the tail matmul+reduce chain runs on little data
            lens = [FCH // 2, FCH // 2, FCH, FCH, FCH,
                    FCH // 2, FCH // 2]
            assert sum(lens) == ROW
            pieces = []
            off = 0
            for ln in lens:
                pieces.append((off, ln))
                off += ln

            for c in range(CH_PER_CORE):
                psum = psum_pool.tile([B, MMB], f32)
                acc_c = acc_pool.tile([P, len(pieces)], f32, tag="acc_c")
                for j, (off, ln) in enumerate(pieces):
                    x = x_pool.tile([P, FCH + 2], f32, tag="x")
                    nc.sync.dma_start(
                        out=x[:, 0:ln + 2], in_=aux[c, :, off: off + ln + 2])
                    if c == 0:
                        # stream the strain mask in slices so chunk 0's
                        # multiply isn't gated on the full 6.5 MB load
                        nc.sync.dma_start(
                            out=ms[:, off:off + ln], in_=msk[:, off:off + ln])
                    u = u_pool.tile([P, FCH], f32, tag="u")
                    # u = max(left, TP, right)
                    nc.vector.scalar_tensor_tensor(
                        u[:, 0:ln], x[:, 0:ln], TP, x[:, 2:ln + 2],
                        op0=Alu.max, op1=Alu.max)
                    ma = ma_pool.tile([P, FCH], bf16, tag="ma")
                    # ma = (u < center) ; acc_c[:, j] = sum(ma)
                    nc.vector.scalar_tensor_tensor(
                        ma[:, 0:ln], u[:, 0:ln], 0.0, x[:, 1:ln + 1],
                        op0=Alu.bypass, op1=Alu.is_lt,
                        accum_out=acc_c[:, j:j + 1])
                    pr = pr_pool.tile([P, FCH], bf16, tag="pr")
                    # pr = ma * ms   (tensor_tensor bf16 -> 2x mode)
                    nc.vector.tensor_tensor(
                        pr[:, 0:ln], ma[:, 0:ln], ms[:, off:off + ln],
                        op=Alu.mult)
                    # inter reduction on TensorE: psum[b, :] += sel.T @ pr
                    nblk = (ln + MMB - 1) // MMB
                    for s in range(nblk):
                        w = min(MMB, ln - s * MMB)
                        nc.tensor.matmul(
                            psum[:, 0:w], sel[:],
                            pr[:, s * MMB:s * MMB + w],
                            start=(j == 0 and s == 0),
                            stop=(j == len(pieces) - 1 and s == nblk - 1))
                nc.vector.tensor_reduce(
                    outt[0:B, 2 * c:2 * c + 1], psum[:],
                    axis=mybir.AxisListType.X, op=Alu.add)
                nc.vector.tensor_reduce(
                    outt[:, 2 * c + 1:2 * c + 2], acc_c[:],
                    axis=mybir.AxisListType.X, op=Alu.add)
                # ship each channel's two columns as soon as they're ready
                nc.sync.dma_start(out=out[:, 2 * c:2 * c + 2],
                                  in_=outt[:, 2 * c:2 * c + 2])

    _prune_redundant_dma_waits(nc, mybir)
    # Bacc compile splits remaining multi-wait instructions (TRN2 allows one
    # sync wait per instruction) via event semaphores, allocs regs, etc.
    nc.compile()
    return nc


def _prune_redundant_dma_waits(nc, mybir):
    """Walrus rejects DMA descriptors with >1 sync wait. Tile (with
    optimize_sems disabled) emits WAR+WAW wait pairs on slot-reusing loads,
    where the WAW wait on the previous DMA's sem is transitively implied by
    the WAR wait (the reader already waited on that DMA). Drop exactly those
    provably-implied DMA-sem waits.

    Soundness: sem S >= v implies every instruction whose update brought S to
    a cumulative value <= v has completed, hence its own waits held. We
    propagate that knowledge (sem -> max implied value) per increment.
    """
    know = {}   # (sem_name, cum_value) -> dict{sem_name: max_value}
    last_ticks = {}  # sem_name -> list of cum values recorded
    cum = {}

    def lookup(sem, val):
        ticks = last_ticks.get(sem)
        if not ticks:
            return {}
        # largest recorded cum tick <= val
        best = None
        for t in ticks:
            if t <= val:
                best = t
            else:
                break
        return know.get((sem, best), {}) if best is not None else {}

    def merge(dst, src):
        for k, v in src.items():
            if dst.get(k, -1) < v:
                dst[k] = v

    insts = [i for b in nc.m.functions[0].blocks for i in b.instructions]
    for inst in insts:
        si = inst.sync_info
        if si is None:
            continue
        waits = list(si.on_wait or [])
        updates = list(si.on_update or [])
        if updates:
            k = {}
            for w in waits:
                if w.wait_value is None:
                    continue
                k[w.ant_name] = max(k.get(w.ant_name, -1), w.wait_value)
                merge(k, lookup(w.ant_name, w.wait_value))
            for u in updates:
                if u.update_value is None:
                    continue
                c = cum.get(u.ant_name, 0) + u.update_value
                cum[u.ant_name] = c
                prev = lookup(u.ant_name, c)
                kk = dict(prev)
                merge(kk, k)
                know[(u.ant_name, c)] = kk
                last_ticks.setdefault(u.ant_name, []).append(c)

    for inst in insts:
        if inst.opcode != "DMACopy":
            continue
        si = inst.sync_info
        if si is None or not si.on_wait or len(si.on_wait) <= 1:
            continue
        waits = list(si.on_wait)
        kept = []
        for i, w in enumerate(waits):
            if not (w.ant_name.startswith("DMASW")
                    or w.ant_name.startswith("DMAHW")):
                kept.append(w)
                continue
            implied = False
            for j, w2 in enumerate(waits):
                if j == i:
                    continue
                if lookup(w2.ant_name, w2.wait_value).get(w.ant_name, -1) \
                        >= w.wait_value:
                    implied = True
                    break
            if not implied:
                kept.append(w)
        # any instruction still multi-wait is split legally by Bacc's
        # generate_event_semaphores during nc.compile()
        inst.sync_info = mybir.SyncInfo(on_wait=kept, on_update=list(si.on_update))


def _get_nc():
    if "nc" not in _CACHE:
        _CACHE["nc"] = _build_bass()
    return _CACHE["nc"]


def _host_strain_mask(qt_strain):
    x = np.ascontiguousarray(qt_strain, dtype=np.float32).reshape(-1)
    m = np.zeros(N, dtype=bool)
    m[1:-1] = (x[1:-1] > x[:-2]) & (x[1:-1] > x[2:]) & (x[1:-1] >= THRESHOLD)
    return m


def _prep_inputs(qt_strain, qt_aux):
    import ml_dtypes
    ms_flat = _host_strain_mask(qt_strain)
    c1 = ms_flat.reshape(B, F * T).sum(axis=1).astype(np.int64)   # [64]
    ms_dev = ms_flat.reshape(P, ROW).astype(ml_dtypes.bfloat16)

    afl = np.ascontiguousarray(qt_aux, dtype=np.float32).reshape(N_AUX, N)
    apad = np.empty((N_AUX, N + 2), np.float32)
    apad[:, 0] = PAD
    apad[:, -1] = PAD
    apad[:, 1:-1] = afl
    sv = np.lib.stride_tricks.as_strided(
        apad, shape=(N_AUX, P, ROW + 2),
        strides=(apad.strides[0], ROW * 4, 4))
    aux_dev = np.ascontiguousarray(sv)     # [32, 128, 25602]
    # sel[p, b] = 1 if partition p belongs to batch b (p in {2b, 2b+1})
    sel_dev = (np.arange(P)[:, None] // 2 ==
               np.arange(B)[None, :]).astype(ml_dtypes.bfloat16)
    in_maps = [
        {"aux": aux_dev[i * CH_PER_CORE:(i + 1) * CH_PER_CORE],
         "msk": ms_dev, "sel": sel_dev}
        for i in range(N_CORES)
    ]
    return in_maps, c1


def _postprocess(results, c1):
    inter = np.empty((N_AUX, B), np.int64)
    c2 = np.empty((N_AUX, B), np.int64)
    for i in range(N_CORES):
        o = np.asarray(results[i]["out"], dtype=np.float64)   # [128, 8]
        for c in range(CH_PER_CORE):
            ch = i * CH_PER_CORE + c
            inter[ch] = np.rint(o[0:B, 2 * c])
            c2[ch] = np.rint(o[:, 2 * c + 1].reshape(B, 2).sum(axis=1))

    interf = inter.astype(np.float32)
    c2f = c2.astype(np.float32)
    c1f = np.broadcast_to(c1.astype(np.float32), (N_AUX, B))
    union = c1f + c2f - interf
    with np.errstate(divide="ignore", invalid="ignore"):
        jac = interf / union
        ratio = interf / c1f
    zero_union = (interf == 0) & (union == 0)
    jac = np.where(zero_union, np.float32(1.0), jac)
    ratio = np.where(zero_union, np.float32(1.0), ratio)
    jac = np.nan_to_num(jac, nan=0.0)
    ratio = np.nan_to_num(ratio, nan=0.0)
    return (jac.reshape(-1).astype(np.float32),
            ratio.reshape(-1).astype(np.float32))


def _run(qt_strain, qt_aux, trace=False, **kw):
    from concourse.bass_utils import run_bass_kernel_spmd
    nc = _get_nc()
    in_maps, c1 = _prep_inputs(qt_strain, qt_aux)
    res = run_bass_kernel_spmd(nc, in_maps, list(range(N_CORES)),
                               trace=trace, **kw)
    return _postprocess(res.results, c1), res


def kernel(qt_strain, qt_aux):
    out, _ = _run(qt_strain, qt_aux, trace=False)
    return out


# revision 25
# speedup vs baseline: 1.3705x; 1.0112x over previous
"""NMS-detection kernel for 8 TRN2 NeuronCores.

Reference computation: per aux channel c (32) and batch b (64), peak masks
(strict local maxima >= 20 over the channel-flattened [B,F,T] array) are
intersected with the strain peak mask; outputs are IoU and inter/|strain|
ratios, flattened to two [2048] vectors.

Strategy:
  - Shard qt_aux along N_aux: 4 channels per core.
  - Host precomputes the strain peak mask (replicated, per the sharding hint)
    as a bf16 0/1 tensor; host does the final tiny [32,64] divisions.
  - Device streams the 52 MB/core of aux data once (memory-bound), computing
    per chunk three 1x DVE ops:
        u   = max(left, THRESH_PRED) max right        (scalar_tensor_tensor)
        ma  = (u bypass) is_lt center, accum -> c2    (scalar_tensor_tensor)
        pr  = (ma mult 1) mult strain_mask, accum -> inter
    The fused accum_out gives per-partition sums for free; partition p covers
    a contiguous 25600-elem span so batch b = partitions {2b, 2b+1}.
"""

import numpy as np

N_AUX = 32
B, F, T = 64, 80, 640
N = B * F * T            # 3,276,800 elems per channel
P = 128                  # SBUF partitions
ROW = N // P             # 25,600 elems per partition row
N_CORES = 8
CH_PER_CORE = N_AUX // N_CORES   # 4
THRESHOLD = 20.0
PAD = np.float32(1.0e30)         # sentinel neighbor: kills first/last peaks
NCHUNK = 5
FCH = ROW // NCHUNK      # 5,120 elems per chunk

_CACHE = {}


def _build_bass():
    from concourse import bacc, mybir, tile

    nc = bacc.Bacc(None)
    f32 = mybir.dt.float32
    bf16 = mybir.dt.bfloat16
    Alu = mybir.AluOpType

    aux = nc.declare_dram_parameter("aux", [CH_PER_CORE, P, ROW + 2], f32,
                                    isOutput=False)
    msk = nc.declare_dram_parameter("msk", [P, ROW], bf16, isOutput=False)
    selp = nc.declare_dram_parameter("sel", [P, B], bf16, isOutput=False)
    out = nc.declare_dram_parameter("out", [P, CH_PER_CORE * 2], f32,
                                    isOutput=True)
    MMB = 512

    # largest f32 strictly below THRESHOLD: (x >= 20.0) == (x > TP)
    TP = float(np.nextafter(np.float32(THRESHOLD), np.float32(0.0)))

    with tile.TileContext(nc) as tc:
        with (
            tc.tile_pool(name="res", bufs=1) as res_pool,
            tc.tile_pool(name="xin", bufs=3) as x_pool,
            tc.tile_pool(name="u", bufs=1) as u_pool,
            tc.tile_pool(name="ma", bufs=1) as ma_pool,
            tc.tile_pool(name="pr", bufs=2) as pr_pool,
            tc.tile_pool(name="acc", bufs=2) as acc_pool,
            tc.tile_pool(name="ps", bufs=2, space="PSUM") as psum_pool,
        ):
            ms = res_pool.tile([P, ROW], bf16)
            sel = res_pool.tile([P, B], bf16)
            nc.sync.dma_start(out=sel[:], in_=selp[:])
            outt = res_pool.tile([P, CH_PER_CORE * 2], f32)

            # per-channel piece grids: channel 0 leads with small pieces so
            # compute starts once ~1.3 MB has landed; the last channel ends
            # with small pieces so the tail matmul+reduce chain is short;
            # middle channels use the minimal op count
            h = FCH // 2
            grids = [
                [h, h, FCH, FCH, FCH, FCH],
                [FCH] * NCHUNK,
                [FCH] * NCHUNK,
                [FCH, FCH, FCH, FCH, h, h],
            ]

            for c in range(CH_PER_CORE):
                lens = grids[c]
                assert sum(lens) == ROW
                pieces = []
                off = 0
                for ln in lens:
                    pieces.append((off, ln))
                    off += ln
                psum = psum_pool.tile([B, MMB], f32)
                acc_c = acc_pool.tile([P, len(pieces)], f32, tag="acc_c")
                for j, (off, ln) in enumerate(pieces):
                    x = x_pool.tile([P, FCH + 2], f32, tag="x")
                    nc.sync.dma_start(
                        out=x[:, 0:ln + 2], in_=aux[c, :, off: off + ln + 2])
                    if c == 0:
                        # stream the strain mask in slices so chunk 0's
                        # multiply isn't gated on the full 6.5 MB load
                        nc.sync.dma_start(
                            out=ms[:, off:off + ln], in_=msk[:, off:off + ln])
                    u = u_pool.tile([P, FCH], f32, tag="u")
                    # u = max(left, TP, right)
                    nc.vector.scalar_tensor_tensor(
                        u[:, 0:ln], x[:, 0:ln], TP, x[:, 2:ln + 2],
                        op0=Alu.max, op1=Alu.max)
                    ma = ma_pool.tile([P, FCH], bf16, tag="ma")
                    # ma = (u < center) ; acc_c[:, j] = sum(ma)
                    nc.vector.scalar_tensor_tensor(
                        ma[:, 0:ln], u[:, 0:ln], 0.0, x[:, 1:ln + 1],
                        op0=Alu.bypass, op1=Alu.is_lt,
                        accum_out=acc_c[:, j:j + 1])
                    pr = pr_pool.tile([P, FCH], bf16, tag="pr")
                    # pr = ma * ms   (tensor_tensor bf16 -> 2x mode)
                    nc.vector.tensor_tensor(
                        pr[:, 0:ln], ma[:, 0:ln], ms[:, off:off + ln],
                        op=Alu.mult)
                    # inter reduction on TensorE: psum[b, :] += sel.T @ pr
                    nblk = (ln + MMB - 1) // MMB
                    for s in range(nblk):
                        w = min(MMB, ln - s * MMB)
                        nc.tensor.matmul(
                            psum[:, 0:w], sel[:],
                            pr[:, s * MMB:s * MMB + w],
                            start=(j == 0 and s == 0),
                            stop=(j == len(pieces) - 1 and s == nblk - 1))
                nc.vector.tensor_reduce(
                    outt[0:B, 2 * c:2 * c + 1], psum[:],
                    axis=mybir.AxisListType.X, op=Alu.add)
                nc.vector.tensor_reduce(
                    outt[:, 2 * c + 1:2 * c + 2], acc_c[:],
                    axis=mybir.AxisListType.X, op=Alu.add)
                # ship each channel's two columns as soon as they're ready
                nc.sync.dma_start(out=out[:, 2 * c:2 * c + 2],
                                  in_=outt[:, 2 * c:2 * c + 2])

    _prune_redundant_dma_waits(nc, mybir)
    # Bacc compile splits remaining multi-wait instructions (TRN2 allows one
    # sync wait per instruction) via event semaphores, allocs regs, etc.
    nc.compile()
    return nc


def _prune_redundant_dma_waits(nc, mybir):
    """Walrus rejects DMA descriptors with >1 sync wait. Tile (with
    optimize_sems disabled) emits WAR+WAW wait pairs on slot-reusing loads,
    where the WAW wait on the previous DMA's sem is transitively implied by
    the WAR wait (the reader already waited on that DMA). Drop exactly those
    provably-implied DMA-sem waits.

    Soundness: sem S >= v implies every instruction whose update brought S to
    a cumulative value <= v has completed, hence its own waits held. We
    propagate that knowledge (sem -> max implied value) per increment.
    """
    know = {}   # (sem_name, cum_value) -> dict{sem_name: max_value}
    last_ticks = {}  # sem_name -> list of cum values recorded
    cum = {}

    def lookup(sem, val):
        ticks = last_ticks.get(sem)
        if not ticks:
            return {}
        # largest recorded cum tick <= val
        best = None
        for t in ticks:
            if t <= val:
                best = t
            else:
                break
        return know.get((sem, best), {}) if best is not None else {}

    def merge(dst, src):
        for k, v in src.items():
            if dst.get(k, -1) < v:
                dst[k] = v

    insts = [i for b in nc.m.functions[0].blocks for i in b.instructions]
    for inst in insts:
        si = inst.sync_info
        if si is None:
            continue
        waits = list(si.on_wait or [])
        updates = list(si.on_update or [])
        if updates:
            k = {}
            for w in waits:
                if w.wait_value is None:
                    continue
                k[w.ant_name] = max(k.get(w.ant_name, -1), w.wait_value)
                merge(k, lookup(w.ant_name, w.wait_value))
            for u in updates:
                if u.update_value is None:
                    continue
                c = cum.get(u.ant_name, 0) + u.update_value
                cum[u.ant_name] = c
                prev = lookup(u.ant_name, c)
                kk = dict(prev)
                merge(kk, k)
                know[(u.ant_name, c)] = kk
                last_ticks.setdefault(u.ant_name, []).append(c)

    for inst in insts:
        if inst.opcode != "DMACopy":
            continue
        si = inst.sync_info
        if si is None or not si.on_wait or len(si.on_wait) <= 1:
            continue
        waits = list(si.on_wait)
        kept = []
        for i, w in enumerate(waits):
            if not (w.ant_name.startswith("DMASW")
                    or w.ant_name.startswith("DMAHW")):
                kept.append(w)
                continue
            implied = False
            for j, w2 in enumerate(waits):
                if j == i:
                    continue
                if lookup(w2.ant_name, w2.wait_value).get(w.ant_name, -1) \
                        >= w.wait_value:
                    implied = True
                    break
            if not implied:
                kept.append(w)
        # any instruction still multi-wait is split legally by Bacc's
        # generate_event_semaphores during nc.compile()
        inst.sync_info = mybir.SyncInfo(on_wait=kept, on_update=list(si.on_update))


def _get_nc():
    if "nc" not in _CACHE:
        _CACHE["nc"] = _build_bass()
    return _CACHE["nc"]


def _host_strain_mask(qt_strain):
    x = np.ascontiguousarray(qt_strain, dtype=np.float32).reshape(-1)
    m = np.zeros(N, dtype=bool)
    m[1:-1] = (x[1:-1] > x[:-2]) & (x[1:-1] > x[2:]) & (x[1:-1] >= THRESHOLD)
    return m


def _prep_inputs(qt_strain, qt_aux):
    import ml_dtypes
    ms_flat = _host_strain_mask(qt_strain)
    c1 = ms_flat.reshape(B, F * T).sum(axis=1).astype(np.int64)   # [64]
    ms_dev = ms_flat.reshape(P, ROW).astype(ml_dtypes.bfloat16)

    afl = np.ascontiguousarray(qt_aux, dtype=np.float32).reshape(N_AUX, N)
    apad = np.empty((N_AUX, N + 2), np.float32)
    apad[:, 0] = PAD
    apad[:, -1] = PAD
    apad[:, 1:-1] = afl
    sv = np.lib.stride_tricks.as_strided(
        apad, shape=(N_AUX, P, ROW + 2),
        strides=(apad.strides[0], ROW * 4, 4))
    aux_dev = np.ascontiguousarray(sv)     # [32, 128, 25602]
    # sel[p, b] = 1 if partition p belongs to batch b (p in {2b, 2b+1})
    sel_dev = (np.arange(P)[:, None] // 2 ==
               np.arange(B)[None, :]).astype(ml_dtypes.bfloat16)
    in_maps = [
        {"aux": aux_dev[i * CH_PER_CORE:(i + 1) * CH_PER_CORE],
         "msk": ms_dev, "sel": sel_dev}
        for i in range(N_CORES)
    ]
    return in_maps, c1


def _postprocess(results, c1):
    inter = np.empty((N_AUX, B), np.int64)
    c2 = np.empty((N_AUX, B), np.int64)
    for i in range(N_CORES):
        o = np.asarray(results[i]["out"], dtype=np.float64)   # [128, 8]
        for c in range(CH_PER_CORE):
            ch = i * CH_PER_CORE + c
            inter[ch] = np.rint(o[0:B, 2 * c])
            c2[ch] = np.rint(o[:, 2 * c + 1].reshape(B, 2).sum(axis=1))

    interf = inter.astype(np.float32)
    c2f = c2.astype(np.float32)
    c1f = np.broadcast_to(c1.astype(np.float32), (N_AUX, B))
    union = c1f + c2f - interf
    with np.errstate(divide="ignore", invalid="ignore"):
        jac = interf / union
        ratio = interf / c1f
    zero_union = (interf == 0) & (union == 0)
    jac = np.where(zero_union, np.float32(1.0), jac)
    ratio = np.where(zero_union, np.float32(1.0), ratio)
    jac = np.nan_to_num(jac, nan=0.0)
    ratio = np.nan_to_num(ratio, nan=0.0)
    return (jac.reshape(-1).astype(np.float32),
            ratio.reshape(-1).astype(np.float32))


def _run(qt_strain, qt_aux, trace=False, **kw):
    from concourse.bass_utils import run_bass_kernel_spmd
    nc = _get_nc()
    in_maps, c1 = _prep_inputs(qt_strain, qt_aux)
    res = run_bass_kernel_spmd(nc, in_maps, list(range(N_CORES)),
                               trace=trace, **kw)
    return _postprocess(res.results, c1), res


def kernel(qt_strain, qt_aux):
    out, _ = _run(qt_strain, qt_aux, trace=False)
    return out
